# revision 25
# baseline (speedup 1.0000x reference)
"""Trainium2 Bass kernel for dynamic per-sample depthwise conv (DPAC).

Reference computation (B=32, C=384, H=W=56, K=7):
  x_avg = mean(x, HW); x_max = max(x, HW)
  x_w   = gelu(x_avg @ w_avg.T + b_avg + x_max @ w_max.T + b_max)
  Gx    = |x_w| * ||w_mix||_row
  s     = x_w * Gx / (mean_c(Gx) + eps)
  kern  = (gamma * w_mix) * s + beta          -> [B,C,7,7]
  out   = depthwise_conv(x, kern, pad=3)

Sharding: pure data parallel, batch split across 8 cores, params replicated.

Implementation notes:
  - Everything on the conv path runs in bf16 (tolerance 2e-2): PE matmuls
    at 1 cyc/row, DVE adds in 2x mode, DVE tensor_scalar in 4x mode, DMA
    bytes halved.
  - Each padded x tile [128, 62, 62] is loaded once and stays resident.
  - The 49 conv taps are split across engines: NPE on the tensor engine as
    diag matmuls (PSUM split into two row-halves so eviction overlaps
    accumulation), NCL as ACT-mul -> DVE-add pairs, NFL as DVE
    tensor_scalar(4x) + add, NEL as fused scalar_tensor_tensor on GPSIMD.
  - Per-tile diag stack is built in one batched tensor_tensor multiply.
  - Pooling + stats of sample b+1 are interleaved into conv of sample b.
  - A post-pass drops every sem wait implied by same-engine program order
    (engines execute in order), removing ~0.5us of bubble per op.
"""

import numpy as np
from contextlib import ExitStack

import concourse.bass as bass
import concourse.tile as tile
from concourse import mybir
from concourse import bass_utils

B, C, H, W, KW = 32, 384, 56, 56, 7
NCORES = 8
BL = B // NCORES            # samples per core
PAD = KW // 2               # 3
HP, WP = H + 2 * PAD, W + 2 * PAD   # 62, 62
P = 128                     # partitions
CG = C // P                 # channel groups (3)
NTAPS = KW * KW             # 49
EPS = 1e-6

# tap assignment: [0, NPE) on PE (contiguous so the diag build can read a
# contiguous kern slice), then ACT->DVE pairs, ACT->Pool pairs, DVE-only
NPE = 30
DL = list(range(NPE, NPE + 4))           # DVE tensor_scalar -> Pool add
CL = list(range(NPE + 4, NPE + 16))      # ACT mul -> DVE add
FL = list(range(NPE + 16, NTAPS))        # DVE tensor_scalar + add
assert len(CL) + len(DL) + len(FL) + NPE == NTAPS

NH = 2                      # PSUM row-halves
RH = H // NH                # 28 rows per half
NCH = 4                     # chunks per half
RCH = RH // NCH             # 7 rows per chunk

F32 = mybir.dt.float32
BF16 = mybir.dt.bfloat16
AX = mybir.AxisListType
OP = mybir.AluOpType
AF = mybir.ActivationFunctionType


KEEP_ONE = True


def _prune_redundant_waits(nc):
    """Drop sem waits that are implied by program order / other waits.

    Engines (DVE/ACT/PE/Pool sequencers and execution) run strictly in
    order, so a wait on one's own engine semaphore is implied by program
    order, and knowledge propagates transitively through cross-engine
    waits. Unlike the conservative variant this prunes all the way to zero
    waits when everything is implied.

    DMA-lane procs are NOT assumed in-order: a lane wait only contributes
    its own fact.
    """
    import bass_rust as _br
    PROC_NAMES = _br.PROC_NAMES
    name_to_idx = {n: i for i, n in enumerate(PROC_NAMES)}
    dma_procs = {i for i, n in enumerate(PROC_NAMES) if n.startswith("DMA")}
    INC = {i: (16 if i in dma_procs else 1) for i in range(len(PROC_NAMES))}

    def sem_proc(ant_name):
        base = ant_name.rsplit("_", 1)[0]
        return name_to_idx.get(base)

    streams = {}
    all_insts = []
    for blk in nc.m.functions[0].blocks:
        for ins in blk.instructions:
            all_insts.append(ins)
            p = ins.bass_scheduled_proc
            t = ins.bass_scheduled_tick
            if p is not None and t is not None:
                streams.setdefault(p, []).append((t, ins))
    for p in streams:
        streams[p].sort(key=lambda x: x[0])

    def merge(a, b):
        for k, v in b.items():
            if a.get(k, -1) < v:
                a[k] = v

    K_memo, SK_memo = {}, {}

    def K(ins):
        r = K_memo.get(ins.name)
        if r is not None:
            return r
        K_memo[ins.name] = {}
        facts = {}
        si = ins.sync_info
        if si is not None:
            for w in si.on_wait:
                if w.wait_mode != "sem-ge-imm" or w.wait_reg is not None:
                    continue
                p = sem_proc(w.ant_name)
                if p is None:
                    continue
                t = w.wait_value // INC[p]
                merge(facts, {p: t})
                if p not in dma_procs:
                    merge(facts, SK(p, t))
                else:
                    merge(facts, DK(p, t))
        K_memo[ins.name] = facts
        return facts

    def SK(p, t):
        st = streams.get(p, [])
        i = 0
        while i < len(st) and st[i][0] <= t:
            i += 1
        key = (p, i)
        r = SK_memo.get(key)
        if r is not None:
            return r
        SK_memo[key] = {}
        if i == 0:
            facts = {}
        else:
            tick_i, ins_i = st[i - 1]
            facts = dict(SK(p, tick_i - 1))
            merge(facts, K(ins_i))
            merge(facts, {p: tick_i})
        SK_memo[key] = facts
        return facts

    def DK(p, t):
        st = streams.get(p, [])
        i = 0
        while i < len(st) and st[i][0] <= t:
            i += 1
        key = ("DK", p, i)
        r = SK_memo.get(key)
        if r is not None:
            return r
        SK_memo[key] = {}
        if i == 0:
            facts = {}
        else:
            tick_i, ins_i = st[i - 1]
            facts = dict(DK(p, tick_i - 1))
            merge(facts, K(ins_i))
        SK_memo[key] = facts
        return facts

    IN_ORDER = ("DVE", "Activation", "PE", "Pool", "SP")

    def prune_inst(ins, strict_one):
        si = ins.sync_info
        if si is None or len(si.on_wait) == 0:
            return 0
        waits = list(si.on_wait)
        if any(w.wait_mode != "sem-ge-imm" or w.wait_reg is not None
               for w in waits):
            if strict_one and len(waits) > 1:
                raise RuntimeError(f"{ins.name}: non-imm wait")
            return 0
        base = {}
        p0, t0 = ins.bass_scheduled_proc, ins.bass_scheduled_tick
        if p0 is not None and t0 is not None and p0 not in dma_procs:
            base = dict(DK(p0, t0 - 1))
            if PROC_NAMES[p0] in IN_ORDER:
                merge(base, SK(p0, t0 - 1))
                merge(base, {p0: t0 - 1})
        kept = list(waits)
        keep_floor = 1 if KEEP_ONE else 0
        changed = True
        while changed and len(kept) > keep_floor:
            changed = False
            for w in list(kept):
                others = [o for o in kept if o is not w]
                facts = dict(base)
                for o in others:
                    p = sem_proc(o.ant_name)
                    if p is None:
                        continue
                    t = o.wait_value // INC[p]
                    merge(facts, {p: t})
                    if p not in dma_procs:
                        merge(facts, SK(p, t))
                    else:
                        merge(facts, DK(p, t))
                pw = sem_proc(w.ant_name)
                tw = w.wait_value // INC[pw] if pw is not None else None
                if pw is not None and facts.get(pw, -1) >= tw:
                    kept = others
                    changed = True
                    break
        if strict_one and len(kept) > 1:
            raise RuntimeError(
                f"{ins.name} ({type(ins).__name__}): cannot reduce waits to "
                "one: " + str([(w.ant_name, w.wait_value) for w in kept]))
        if len(kept) < len(waits):
            si.on_wait = kept
            ins.sync_info = si
            return 1
        return 0

    n_pruned = 0
    for ins in all_insts:
        if not ins.is_executable():
            continue
        n_pruned += prune_inst(ins, strict_one=not ins.is_sequencer_only())
    return n_pruned


def _build_bass():
    nc = bass.Bass("TRN2", target_bir_lowering=False, debug=False,
                   num_devices=NCORES)

    xp = nc.dram_tensor("xp", [BL, C, HP, WP], BF16, kind="ExternalInput").ap()
    w2t = nc.dram_tensor("w2t", [P, 2, CG, C], F32, kind="ExternalInput").ap()
    bsum = nc.dram_tensor("bsum", [P, CG], F32, kind="ExternalInput").ap()
    gw = nc.dram_tensor("gw", [P, CG, NTAPS], F32, kind="ExternalInput").ap()
    wn = nc.dram_tensor("wn", [P, CG], F32, kind="ExternalInput").ap()
    bet = nc.dram_tensor("bet", [P, CG], F32, kind="ExternalInput").ap()
    istk = nc.dram_tensor("istk", [P, P], BF16, kind="ExternalInput").ap()
    outd = nc.dram_tensor("out", [BL, C, H, W], BF16, kind="ExternalOutput").ap()

    with tile.TileContext(nc) as tc, ExitStack() as ctx:
        sing = ctx.enter_context(tc.tile_pool(name="sing", bufs=1))
        dstkp = ctx.enter_context(tc.tile_pool(name="dstkp", bufs=2))
        accp = ctx.enter_context(tc.tile_pool(name="accp", bufs=2))
        accgp = ctx.enter_context(tc.tile_pool(name="accgp", bufs=1))
        evp = ctx.enter_context(tc.tile_pool(name="evp", bufs=2))
        tmpp = ctx.enter_context(tc.tile_pool(name="tmpp", bufs=3))
        dtmpp = ctx.enter_context(tc.tile_pool(name="dtmpp", bufs=4))
        tmp2p = ctx.enter_context(tc.tile_pool(name="tmp2p", bufs=1))
        smallp = ctx.enter_context(tc.tile_pool(name="smallp", bufs=2))
        pep = ctx.enter_context(tc.tile_pool(name="pep", bufs=2, space="PSUM"))

        # ---- params ----
        w2_sb = sing.tile([P, 2, CG, C], F32)
        nc.sync.dma_start(out=w2_sb, in_=w2t)
        bs_sb = sing.tile([P, CG], F32)
        nc.sync.dma_start(out=bs_sb, in_=bsum)
        gw_sb = sing.tile([P, CG, NTAPS], F32)
        nc.sync.dma_start(out=gw_sb, in_=gw)
        wn_sb = sing.tile([P, CG], F32)
        nc.sync.dma_start(out=wn_sb, in_=wn)
        bet_sb = sing.tile([P, CG], F32)
        nc.sync.dma_start(out=bet_sb, in_=bet)
        istk_sb = sing.tile([P, P], BF16)
        nc.sync.dma_start(out=istk_sb, in_=istk)

        ones_col = sing.tile([P, 1], F32)
        nc.vector.memset(ones_col, 1.0)
        ones_row = sing.tile([1, P], F32)
        nc.vector.memset(ones_row, 1.0)

        # observers: thread param-load completion into each engine's
        # program-order knowledge so later waits collapse to one sem
        obs_a = sing.tile([P, 3], F32)
        nc.scalar.copy(out=obs_a[:, 0:1], in_=w2_sb[:, 0, 0, 0:1])
        nc.scalar.copy(out=obs_a[:, 1:2], in_=bs_sb[:, 0:1])
        nc.scalar.copy(out=obs_a[:, 2:3], in_=wn_sb[:, 0:1])
        obs_v = sing.tile([P, 3], F32)
        nc.vector.tensor_copy(out=obs_v[:, 0:1], in_=gw_sb[:, 0, 0:1])
        nc.vector.tensor_copy(out=obs_v[:, 1:2], in_=bet_sb[:, 0:1])
        nc.vector.tensor_copy(out=obs_v[:, 2:3], in_=istk_sb[:, 0:1])

        # ---- x tiles (all resident) ----
        xts = []
        for b in range(BL):
            for g in range(CG):
                xt = sing.tile([P, HP, WP], BF16, tag=f"xt{b}_{g}")
                nc.sync.dma_start(out=xt, in_=xp[b, g * P:(g + 1) * P, :, :])
                xts.append(xt)

        x2 = sing.tile([P, 2, CG, BL], F32)      # [mean-sum | max]
        xw = sing.tile([P, CG, BL], F32)
        gx = sing.tile([P, CG, BL], F32)
        s3 = sing.tile([P, CG, BL], F32)
        kern = sing.tile([P, CG, BL, NTAPS], F32)
        rb_sb = sing.tile([P, BL], F32)

        def pool_tile(b, g):
            xt = xts[b * CG + g]
            # spatial sum via ACT accumulator (padding zeros are harmless);
            # 1/(H*W) is folded into w_avg host-side
            nc.scalar.activation(out=xt, in_=xt, func=AF.Copy,
                                 accum_out=x2[:, 0, g, b:b + 1])
            # max over the interior (max of 3136 randn values is > 0, so the
            # padded zeros can never win; use the strided interior view)
            nc.vector.tensor_reduce(
                out=x2[:, 1, g, b:b + 1],
                in_=xt[:, PAD:PAD + H, PAD:PAD + W], axis=AX.XY, op=OP.max)

        def stats(b):
            pst = pep.tile([P, NCH, 512], F32, tag="peacc")
            for m in range(CG):
                k = 0
                for s in range(2):
                    for g in range(CG):
                        nc.tensor.matmul(
                            pst[:, 0, m:m + 1],
                            w2_sb[:, s, g, m * P:(m + 1) * P],
                            x2[:, s, g, b:b + 1],
                            start=(k == 0), stop=(k == 5))
                        k += 1
            for m in range(CG):
                nc.scalar.activation(out=xw[:, m, b:b + 1],
                                     in_=pst[:, 0, m:m + 1], func=AF.Gelu,
                                     bias=bs_sb[:, m:m + 1], scale=1.0)
                nc.scalar.activation(out=gx[:, m, b:b + 1],
                                     in_=xw[:, m, b:b + 1], func=AF.Abs,
                                     scale=wn_sb[:, m:m + 1])
            # channel sum of Gx via PE (sums partitions), then r = 1/(mean+eps)
            nc.tensor.matmul(pst[0:1, 1, 0:CG], ones_col, gx[:, :, b],
                             start=True, stop=True)
            r1 = smallp.tile([1, 1], F32, tag="r1")
            nc.vector.tensor_reduce(out=r1, in_=pst[0:1, 1, 0:CG],
                                    axis=AX.X, op=OP.add)
            nc.vector.tensor_scalar(out=r1, in0=r1, scalar1=1.0 / C,
                                    scalar2=EPS, op0=OP.mult, op1=OP.add)
            nc.vector.reciprocal(out=r1, in_=r1)
            nc.tensor.matmul(pst[:, 2, 0:1], ones_row, r1,
                             start=True, stop=True)
            nc.vector.tensor_copy(out=rb_sb[:, b:b + 1], in_=pst[:, 2, 0:1])
            # s = xw * gx * r ; kern = gw * s + beta
            nc.vector.tensor_mul(out=s3[:, :, b], in0=xw[:, :, b],
                                 in1=gx[:, :, b])
            nc.vector.tensor_scalar_mul(s3[:, :, b], s3[:, :, b],
                                        rb_sb[:, b:b + 1])
            for g in range(CG):
                nc.vector.tensor_scalar(
                    out=kern[:, g, b, :], in0=gw_sb[:, g, :],
                    scalar1=s3[:, g, b:b + 1], scalar2=bet_sb[:, g:g + 1],
                    op0=OP.mult, op1=OP.add)

        def conv_tile(b, g):
            xt = xts[b * CG + g]
            ks = kern[:, g, b, :]

            # diag stack: dstk[:, i, :] = I * kern[., tap i]  (one batched op;
            # the identity is a stride-0 broadcast view)
            dstk = dstkp.tile([P, NPE, P], BF16)
            kb = ks[:, 0:NPE].unsqueeze(2).broadcast_to([P, NPE, P])
            ib = istk_sb.unsqueeze(1).broadcast_to([P, NPE, P])
            nc.vector.tensor_mul(out=dstk, in0=ib, in1=kb)

            # DVE tensor_scalar muls feeding the Pool accumulation chain
            # (all Pool/ACT waits then reference only the DVE semaphore)
            dtmps = []
            for t in DL:
                di, dj = divmod(t, KW)
                dtmp = dtmpp.tile([P, H, W], BF16, tag="t")
                nc.vector.tensor_scalar(
                    out=dtmp, in0=xt[:, di:di + H, dj:dj + W],
                    scalar1=ks[:, t:t + 1], scalar2=None, op0=OP.mult)
                dtmps.append(dtmp)

            # PE: NPE taps as diag matmuls, two row-halves for double-buffer
            paccs = []
            for h in range(NH):
                pacc = pep.tile([P, NCH, 512], F32, tag="peacc")
                for c4 in range(NCH):
                    r0 = h * RH + c4 * RCH
                    for ti in range(NPE):
                        di, dj = divmod(ti, KW)
                        nc.tensor.matmul(
                            pacc[:, c4, 0:RCH * W], dstk[:, ti, :],
                            xt[:, r0 + di:r0 + di + RCH, dj:dj + W],
                            start=(ti == 0), stop=(ti == NPE - 1))
                paccs.append(pacc)

            # ACT: mul taps. Pool-destined taps first (its chain is slow);
            # evictions interleaved so PSUM frees early.
            ev = evp.tile([P, H, W], BF16, tag="ev")
            tmps = []
            for i, t in enumerate(CL):
                di, dj = divmod(t, KW)
                tmp = tmpp.tile([P, H, W], BF16, tag="t")
                nc.scalar.mul(out=tmp, in_=xt[:, di:di + H, dj:dj + W],
                              mul=ks[:, t:t + 1])
                tmps.append(tmp)
                if i == 5:
                    nc.scalar.copy(
                        out=ev[:, 0:RH, :].rearrange(
                            "p (a b) w -> p a (b w)", a=NCH),
                        in_=paccs[0][:, :, 0:RCH * W])
                elif i == 10:
                    nc.scalar.copy(
                        out=ev[:, RH:H, :].rearrange(
                            "p (a b) w -> p a (b w)", a=NCH),
                        in_=paccs[1][:, :, 0:RCH * W])

            # Pool: accumulate its tmps
            accg = accgp.tile([P, H, W], BF16, tag="accg")
            nc.gpsimd.tensor_copy(out=accg, in_=dtmps[0])
            for tmp in dtmps[1:]:
                nc.gpsimd.tensor_add(out=accg, in0=accg, in1=tmp)

            # DVE: tensor_scalar(4x) + add taps, then accumulate ACT tmps
            acc = accp.tile([P, H, W], BF16, tag="acc")
            t0 = FL[0]
            di, dj = divmod(t0, KW)
            nc.vector.tensor_scalar(out=acc, in0=xt[:, di:di + H, dj:dj + W],
                                    scalar1=ks[:, t0:t0 + 1], scalar2=None,
                                    op0=OP.mult)
            for t in FL[1:]:
                di, dj = divmod(t, KW)
                tmp2 = tmp2p.tile([P, H, W], BF16, tag="tmp2")
                nc.vector.tensor_scalar(
                    out=tmp2, in0=xt[:, di:di + H, dj:dj + W],
                    scalar1=ks[:, t:t + 1], scalar2=None, op0=OP.mult)
                nc.vector.tensor_add(out=acc, in0=acc, in1=tmp2)
            for tmp in tmps:
                nc.vector.tensor_add(out=acc, in0=acc, in1=tmp)
            nc.vector.tensor_add(out=acc, in0=acc, in1=ev)
            nc.vector.tensor_add(out=acc, in0=acc, in1=accg)

            nc.sync.dma_start(out=outd[b, g * P:(g + 1) * P, :, :], in_=acc)
            # DVE observer of the store completion: collapses the final
            # drain's (and slot-recycling) waits onto the DVE semaphore
            nc.vector.memset(acc[:, 0, 0:1], 0.0)

        # ---- schedule: pool/stats of b+1 interleaved into conv of b ----
        for g in range(CG):
            pool_tile(0, g)
        stats(0)
        for b in range(BL):
            for g in range(CG):
                if b + 1 < BL:
                    pool_tile(b + 1, g)
                    if g == CG - 1:
                        stats(b + 1)
                conv_tile(b, g)

    import sys
    sys.setrecursionlimit(100000)
    _prune_redundant_waits(nc)
    return nc


_NC_CACHE = {}


def _get_nc():
    if "nc" not in _NC_CACHE:
        _NC_CACHE["nc"] = _build_bass()
    return _NC_CACHE["nc"]


def _prep_inputs(x, w_avg, b_avg, w_max, b_max, w_mix, gamma, beta):
    import ml_dtypes
    bf16 = ml_dtypes.bfloat16

    x = np.asarray(x, dtype=np.float32)
    xpad = np.zeros((B, C, HP, WP), dtype=np.float32)
    xpad[:, :, PAD:PAD + H, PAD:PAD + W] = x
    xpad = xpad.astype(bf16)

    w_avg = np.asarray(w_avg, np.float32) / (H * W)   # fold the mean
    w_max = np.asarray(w_max, np.float32)
    # stationary layout [P, 2, CG, C]: w2t[p, s, g, :] = W_s[:, g*P+p]
    w2t = np.stack([
        w_avg.T.reshape(CG, P, C).transpose(1, 0, 2),
        w_max.T.reshape(CG, P, C).transpose(1, 0, 2),
    ], axis=1)
    bsum = (np.asarray(b_avg, np.float32) + np.asarray(b_max, np.float32))
    w_mix = np.asarray(w_mix, np.float32)
    gamma = np.asarray(gamma, np.float32).reshape(C)
    beta = np.asarray(beta, np.float32).reshape(C)
    gw = (gamma[:, None] * w_mix).reshape(CG, P, NTAPS).transpose(1, 0, 2)
    wn = np.sqrt((w_mix * w_mix).sum(axis=1)).reshape(CG, P).T
    istk = np.eye(P, dtype=np.float32).astype(bf16)

    shared = {
        "w2t": np.ascontiguousarray(w2t),
        "bsum": np.ascontiguousarray(bsum.reshape(CG, P).T),
        "gw": np.ascontiguousarray(gw),
        "wn": np.ascontiguousarray(wn),
        "bet": np.ascontiguousarray(beta.reshape(CG, P).T),
        "istk": istk,
    }
    in_maps = []
    for i in range(NCORES):
        m = dict(shared)
        m["xp"] = np.ascontiguousarray(xpad[i * BL:(i + 1) * BL])
        in_maps.append(m)
    return in_maps


def run(inputs, trace=False):
    nc = _get_nc()
    in_maps = _prep_inputs(**inputs)
    res = bass_utils.run_bass_kernel_spmd(
        nc, in_maps, core_ids=list(range(NCORES)), trace=trace)
    outs = [np.asarray(res.results[i]["out"]) for i in range(NCORES)]
    full = np.concatenate(outs, axis=0).astype(np.float32)
    return full, res


def kernel(**inputs) -> np.ndarray:
    full, _ = run(inputs, trace=False)
    return full


# revision 26
# speedup vs baseline: 1.0087x; 1.0087x over previous
"""Trainium2 Bass kernel for dynamic per-sample depthwise conv (DPAC).

Reference computation (B=32, C=384, H=W=56, K=7):
  x_avg = mean(x, HW); x_max = max(x, HW)
  x_w   = gelu(x_avg @ w_avg.T + b_avg + x_max @ w_max.T + b_max)
  Gx    = |x_w| * ||w_mix||_row
  s     = x_w * Gx / (mean_c(Gx) + eps)
  kern  = (gamma * w_mix) * s + beta          -> [B,C,7,7]
  out   = depthwise_conv(x, kern, pad=3)

Sharding: pure data parallel, batch split across 8 cores, params replicated.

Implementation notes:
  - Everything on the conv path runs in bf16 (tolerance 2e-2): PE matmuls
    at 1 cyc/row, DVE adds in 2x mode, DVE tensor_scalar in 4x mode, DMA
    bytes halved.
  - Each padded x tile [128, 62, 62] is loaded once and stays resident.
  - The 49 conv taps are split across engines: NPE on the tensor engine as
    diag matmuls (PSUM split into two row-halves so eviction overlaps
    accumulation), NCL as ACT-mul -> DVE-add pairs, NFL as DVE
    tensor_scalar(4x) + add, NEL as fused scalar_tensor_tensor on GPSIMD.
  - Per-tile diag stack is built in one batched tensor_tensor multiply.
  - Pooling + stats of sample b+1 are interleaved into conv of sample b.
  - A post-pass drops every sem wait implied by same-engine program order
    (engines execute in order), removing ~0.5us of bubble per op.
"""

import numpy as np
from contextlib import ExitStack

import concourse.bass as bass
import concourse.tile as tile
from concourse import mybir
from concourse import bass_utils

B, C, H, W, KW = 32, 384, 56, 56, 7
NCORES = 8
BL = B // NCORES            # samples per core
PAD = KW // 2               # 3
HP, WP = H + 2 * PAD, W + 2 * PAD   # 62, 62
P = 128                     # partitions
CG = C // P                 # channel groups (3)
NTAPS = KW * KW             # 49
EPS = 1e-6

# tap assignment: [0, NPE) on PE (contiguous so the diag build can read a
# contiguous kern slice), then ACT->DVE pairs, ACT->Pool pairs, DVE-only
NPE = 30
DL = list(range(NPE, NPE + 4))           # DVE tensor_scalar -> Pool add
CL = list(range(NPE + 4, NPE + 16))      # ACT mul -> DVE add
FL = list(range(NPE + 16, NTAPS))        # DVE tensor_scalar + add
assert len(CL) + len(DL) + len(FL) + NPE == NTAPS

NH = 2                      # PSUM row-halves
RH = H // NH                # 28 rows per half
NCH = 4                     # chunks per half
RCH = RH // NCH             # 7 rows per chunk

F32 = mybir.dt.float32
BF16 = mybir.dt.bfloat16
AX = mybir.AxisListType
OP = mybir.AluOpType
AF = mybir.ActivationFunctionType


KEEP_ONE = True


def _prune_redundant_waits(nc):
    """Drop sem waits that are implied by program order / other waits.

    Engines (DVE/ACT/PE/Pool sequencers and execution) run strictly in
    order, so a wait on one's own engine semaphore is implied by program
    order, and knowledge propagates transitively through cross-engine
    waits. Unlike the conservative variant this prunes all the way to zero
    waits when everything is implied.

    DMA-lane procs are NOT assumed in-order: a lane wait only contributes
    its own fact.
    """
    import bass_rust as _br
    PROC_NAMES = _br.PROC_NAMES
    name_to_idx = {n: i for i, n in enumerate(PROC_NAMES)}
    dma_procs = {i for i, n in enumerate(PROC_NAMES) if n.startswith("DMA")}
    INC = {i: (16 if i in dma_procs else 1) for i in range(len(PROC_NAMES))}

    def sem_proc(ant_name):
        base = ant_name.rsplit("_", 1)[0]
        return name_to_idx.get(base)

    streams = {}
    all_insts = []
    for blk in nc.m.functions[0].blocks:
        for ins in blk.instructions:
            all_insts.append(ins)
            p = ins.bass_scheduled_proc
            t = ins.bass_scheduled_tick
            if p is not None and t is not None:
                streams.setdefault(p, []).append((t, ins))
    for p in streams:
        streams[p].sort(key=lambda x: x[0])

    def merge(a, b):
        for k, v in b.items():
            if a.get(k, -1) < v:
                a[k] = v

    K_memo, SK_memo = {}, {}

    def K(ins):
        r = K_memo.get(ins.name)
        if r is not None:
            return r
        K_memo[ins.name] = {}
        facts = {}
        si = ins.sync_info
        if si is not None:
            for w in si.on_wait:
                if w.wait_mode != "sem-ge-imm" or w.wait_reg is not None:
                    continue
                p = sem_proc(w.ant_name)
                if p is None:
                    continue
                t = w.wait_value // INC[p]
                merge(facts, {p: t})
                if p not in dma_procs:
                    merge(facts, SK(p, t))
                else:
                    merge(facts, DK(p, t))
        K_memo[ins.name] = facts
        return facts

    def SK(p, t):
        st = streams.get(p, [])
        i = 0
        while i < len(st) and st[i][0] <= t:
            i += 1
        key = (p, i)
        r = SK_memo.get(key)
        if r is not None:
            return r
        SK_memo[key] = {}
        if i == 0:
            facts = {}
        else:
            tick_i, ins_i = st[i - 1]
            facts = dict(SK(p, tick_i - 1))
            merge(facts, K(ins_i))
            merge(facts, {p: tick_i})
        SK_memo[key] = facts
        return facts

    def DK(p, t):
        st = streams.get(p, [])
        i = 0
        while i < len(st) and st[i][0] <= t:
            i += 1
        key = ("DK", p, i)
        r = SK_memo.get(key)
        if r is not None:
            return r
        SK_memo[key] = {}
        if i == 0:
            facts = {}
        else:
            tick_i, ins_i = st[i - 1]
            facts = dict(DK(p, tick_i - 1))
            merge(facts, K(ins_i))
        SK_memo[key] = facts
        return facts

    IN_ORDER = ("DVE", "Activation", "PE", "Pool", "SP")

    def prune_inst(ins, strict_one):
        si = ins.sync_info
        if si is None or len(si.on_wait) == 0:
            return 0
        waits = list(si.on_wait)
        if any(w.wait_mode != "sem-ge-imm" or w.wait_reg is not None
               for w in waits):
            if strict_one and len(waits) > 1:
                raise RuntimeError(f"{ins.name}: non-imm wait")
            return 0
        base = {}
        p0, t0 = ins.bass_scheduled_proc, ins.bass_scheduled_tick
        if p0 is not None and t0 is not None and p0 not in dma_procs:
            base = dict(DK(p0, t0 - 1))
            if PROC_NAMES[p0] in IN_ORDER:
                merge(base, SK(p0, t0 - 1))
                merge(base, {p0: t0 - 1})
        kept = list(waits)
        keep_floor = 1 if KEEP_ONE else 0
        changed = True
        while changed and len(kept) > keep_floor:
            changed = False
            for w in list(kept):
                others = [o for o in kept if o is not w]
                facts = dict(base)
                for o in others:
                    p = sem_proc(o.ant_name)
                    if p is None:
                        continue
                    t = o.wait_value // INC[p]
                    merge(facts, {p: t})
                    if p not in dma_procs:
                        merge(facts, SK(p, t))
                    else:
                        merge(facts, DK(p, t))
                pw = sem_proc(w.ant_name)
                tw = w.wait_value // INC[pw] if pw is not None else None
                if pw is not None and facts.get(pw, -1) >= tw:
                    kept = others
                    changed = True
                    break
        if strict_one and len(kept) > 1:
            raise RuntimeError(
                f"{ins.name} ({type(ins).__name__}): cannot reduce waits to "
                "one: " + str([(w.ant_name, w.wait_value) for w in kept]))
        if len(kept) < len(waits):
            si.on_wait = kept
            ins.sync_info = si
            return 1
        return 0

    n_pruned = 0
    for ins in all_insts:
        if not ins.is_executable():
            continue
        n_pruned += prune_inst(ins, strict_one=not ins.is_sequencer_only())
    return n_pruned


def _build_bass():
    nc = bass.Bass("TRN2", target_bir_lowering=False, debug=False,
                   num_devices=NCORES)

    xp = nc.dram_tensor("xp", [BL, C, HP, WP], BF16, kind="ExternalInput").ap()
    w2t = nc.dram_tensor("w2t", [P, 2, CG, C], F32, kind="ExternalInput").ap()
    bsum = nc.dram_tensor("bsum", [P, CG], F32, kind="ExternalInput").ap()
    gw = nc.dram_tensor("gw", [P, CG, NTAPS], F32, kind="ExternalInput").ap()
    wn = nc.dram_tensor("wn", [P, CG], F32, kind="ExternalInput").ap()
    bet = nc.dram_tensor("bet", [P, CG], F32, kind="ExternalInput").ap()
    istk = nc.dram_tensor("istk", [P, P], BF16, kind="ExternalInput").ap()
    outd = nc.dram_tensor("out", [BL, C, H, W], BF16, kind="ExternalOutput").ap()

    with tile.TileContext(nc) as tc, ExitStack() as ctx:
        sing = ctx.enter_context(tc.tile_pool(name="sing", bufs=1))
        dstkp = ctx.enter_context(tc.tile_pool(name="dstkp", bufs=2))
        accp = ctx.enter_context(tc.tile_pool(name="accp", bufs=2))
        accgp = ctx.enter_context(tc.tile_pool(name="accgp", bufs=1))
        evp = ctx.enter_context(tc.tile_pool(name="evp", bufs=2))
        tmpp = ctx.enter_context(tc.tile_pool(name="tmpp", bufs=3))
        dtmpp = ctx.enter_context(tc.tile_pool(name="dtmpp", bufs=4))
        tmp2p = ctx.enter_context(tc.tile_pool(name="tmp2p", bufs=1))
        smallp = ctx.enter_context(tc.tile_pool(name="smallp", bufs=2))
        pep = ctx.enter_context(tc.tile_pool(name="pep", bufs=2, space="PSUM"))

        # ---- params ----
        w2_sb = sing.tile([P, 2, CG, C], F32)
        nc.sync.dma_start(out=w2_sb, in_=w2t)
        bs_sb = sing.tile([P, CG], F32)
        nc.sync.dma_start(out=bs_sb, in_=bsum)
        gw_sb = sing.tile([P, CG, NTAPS], F32)
        nc.sync.dma_start(out=gw_sb, in_=gw)
        wn_sb = sing.tile([P, CG], F32)
        nc.sync.dma_start(out=wn_sb, in_=wn)
        bet_sb = sing.tile([P, CG], F32)
        nc.sync.dma_start(out=bet_sb, in_=bet)
        istk_sb = sing.tile([P, P], BF16)
        nc.sync.dma_start(out=istk_sb, in_=istk)

        ones_col = sing.tile([P, 1], F32)
        nc.vector.memset(ones_col, 1.0)
        ones_row = sing.tile([1, P], F32)
        nc.vector.memset(ones_row, 1.0)

        # observers: thread param-load completion into each engine's
        # program-order knowledge so later waits collapse to one sem
        obs_a = sing.tile([P, 3], F32)
        nc.scalar.copy(out=obs_a[:, 0:1], in_=w2_sb[:, 0, 0, 0:1])
        nc.scalar.copy(out=obs_a[:, 1:2], in_=bs_sb[:, 0:1])
        nc.scalar.copy(out=obs_a[:, 2:3], in_=wn_sb[:, 0:1])
        obs_v = sing.tile([P, 3], F32)
        nc.vector.tensor_copy(out=obs_v[:, 0:1], in_=gw_sb[:, 0, 0:1])
        nc.vector.tensor_copy(out=obs_v[:, 1:2], in_=bet_sb[:, 0:1])
        nc.vector.tensor_copy(out=obs_v[:, 2:3], in_=istk_sb[:, 0:1])

        # ---- x tiles (all resident) ----
        xts = []
        for b in range(BL):
            for g in range(CG):
                xt = sing.tile([P, HP, WP], BF16, tag=f"xt{b}_{g}")
                nc.sync.dma_start(out=xt, in_=xp[b, g * P:(g + 1) * P, :, :])
                xts.append(xt)

        x2 = sing.tile([P, 2, CG, BL], F32)      # [mean-sum | max]
        xw = sing.tile([P, CG, BL], F32)
        gx = sing.tile([P, CG, BL], F32)
        s3 = sing.tile([P, CG, BL], F32)
        kern = sing.tile([P, CG, BL, NTAPS], F32)
        rb_sb = sing.tile([P, BL], F32)

        def pool_tile(b, g):
            xt = xts[b * CG + g]
            # spatial sum via ACT accumulator (padding zeros are harmless);
            # 1/(H*W) is folded into w_avg host-side
            nc.scalar.activation(out=xt, in_=xt, func=AF.Copy,
                                 accum_out=x2[:, 0, g, b:b + 1])
            # max over the interior (max of 3136 randn values is > 0, so the
            # padded zeros can never win; use the strided interior view)
            nc.vector.tensor_reduce(
                out=x2[:, 1, g, b:b + 1],
                in_=xt[:, PAD:PAD + H, PAD:PAD + W], axis=AX.XY, op=OP.max)

        def stats(b):
            pst = pep.tile([P, NCH, 512], F32, tag="peacc")
            for m in range(CG):
                k = 0
                for s in range(2):
                    for g in range(CG):
                        nc.tensor.matmul(
                            pst[:, 0, m:m + 1],
                            w2_sb[:, s, g, m * P:(m + 1) * P],
                            x2[:, s, g, b:b + 1],
                            start=(k == 0), stop=(k == 5))
                        k += 1
            for m in range(CG):
                nc.scalar.activation(out=xw[:, m, b:b + 1],
                                     in_=pst[:, 0, m:m + 1], func=AF.Gelu,
                                     bias=bs_sb[:, m:m + 1], scale=1.0)
                nc.scalar.activation(out=gx[:, m, b:b + 1],
                                     in_=xw[:, m, b:b + 1], func=AF.Abs,
                                     scale=wn_sb[:, m:m + 1])
            # channel sum of Gx via PE (sums partitions), then r = 1/(mean+eps)
            nc.tensor.matmul(pst[0:1, 1, 0:CG], ones_col, gx[:, :, b],
                             start=True, stop=True)
            r1 = smallp.tile([1, 1], F32, tag="r1")
            nc.vector.tensor_reduce(out=r1, in_=pst[0:1, 1, 0:CG],
                                    axis=AX.X, op=OP.add)
            nc.vector.tensor_scalar(out=r1, in0=r1, scalar1=1.0 / C,
                                    scalar2=EPS, op0=OP.mult, op1=OP.add)
            nc.vector.reciprocal(out=r1, in_=r1)
            nc.tensor.matmul(pst[:, 2, 0:1], ones_row, r1,
                             start=True, stop=True)
            nc.vector.tensor_copy(out=rb_sb[:, b:b + 1], in_=pst[:, 2, 0:1])
            # s = xw * gx * r ; kern = gw * s + beta
            nc.vector.tensor_mul(out=s3[:, :, b], in0=xw[:, :, b],
                                 in1=gx[:, :, b])
            nc.vector.tensor_scalar_mul(s3[:, :, b], s3[:, :, b],
                                        rb_sb[:, b:b + 1])
            for g in range(CG):
                nc.vector.tensor_scalar(
                    out=kern[:, g, b, :], in0=gw_sb[:, g, :],
                    scalar1=s3[:, g, b:b + 1], scalar2=bet_sb[:, g:g + 1],
                    op0=OP.mult, op1=OP.add)

        def conv_tile(b, g, filler=None):
            xt = xts[b * CG + g]
            ks = kern[:, g, b, :]

            # diag stack: dstk[:, i, :] = I * kern[., tap i]  (one batched op;
            # the identity is a stride-0 broadcast view)
            dstk = dstkp.tile([P, NPE, P], BF16)
            kb = ks[:, 0:NPE].unsqueeze(2).broadcast_to([P, NPE, P])
            ib = istk_sb.unsqueeze(1).broadcast_to([P, NPE, P])
            nc.vector.tensor_mul(out=dstk, in0=ib, in1=kb)

            # DVE tensor_scalar muls feeding the Pool accumulation chain
            # (all Pool/ACT waits then reference only the DVE semaphore)
            dtmps = []
            for t in DL:
                di, dj = divmod(t, KW)
                dtmp = dtmpp.tile([P, H, W], BF16, tag="t")
                nc.vector.tensor_scalar(
                    out=dtmp, in0=xt[:, di:di + H, dj:dj + W],
                    scalar1=ks[:, t:t + 1], scalar2=None, op0=OP.mult)
                dtmps.append(dtmp)

            # PE: NPE taps as diag matmuls, two row-halves for double-buffer
            paccs = []
            for h in range(NH):
                pacc = pep.tile([P, NCH, 512], F32, tag="peacc")
                for c4 in range(NCH):
                    r0 = h * RH + c4 * RCH
                    for ti in range(NPE):
                        di, dj = divmod(ti, KW)
                        nc.tensor.matmul(
                            pacc[:, c4, 0:RCH * W], dstk[:, ti, :],
                            xt[:, r0 + di:r0 + di + RCH, dj:dj + W],
                            start=(ti == 0), stop=(ti == NPE - 1))
                paccs.append(pacc)

            # ACT: mul taps. Pool-destined taps first (its chain is slow);
            # evictions interleaved so PSUM frees early.
            ev = evp.tile([P, H, W], BF16, tag="ev")
            tmps = []
            for i, t in enumerate(CL):
                di, dj = divmod(t, KW)
                tmp = tmpp.tile([P, H, W], BF16, tag="t")
                nc.scalar.mul(out=tmp, in_=xt[:, di:di + H, dj:dj + W],
                              mul=ks[:, t:t + 1])
                tmps.append(tmp)
                if i == 5:
                    nc.scalar.copy(
                        out=ev[:, 0:RH, :].rearrange(
                            "p (a b) w -> p a (b w)", a=NCH),
                        in_=paccs[0][:, :, 0:RCH * W])
            # next-sample pooling/stats fill ACT's wait for PE's second half
            if filler is not None:
                filler()
            nc.scalar.copy(
                out=ev[:, RH:H, :].rearrange(
                    "p (a b) w -> p a (b w)", a=NCH),
                in_=paccs[1][:, :, 0:RCH * W])

            # Pool: accumulate its tmps
            accg = accgp.tile([P, H, W], BF16, tag="accg")
            nc.gpsimd.tensor_copy(out=accg, in_=dtmps[0])
            for tmp in dtmps[1:]:
                nc.gpsimd.tensor_add(out=accg, in0=accg, in1=tmp)

            # DVE: tensor_scalar(4x) + add taps, then accumulate ACT tmps
            acc = accp.tile([P, H, W], BF16, tag="acc")
            t0 = FL[0]
            di, dj = divmod(t0, KW)
            nc.vector.tensor_scalar(out=acc, in0=xt[:, di:di + H, dj:dj + W],
                                    scalar1=ks[:, t0:t0 + 1], scalar2=None,
                                    op0=OP.mult)
            for t in FL[1:]:
                di, dj = divmod(t, KW)
                tmp2 = tmp2p.tile([P, H, W], BF16, tag="tmp2")
                nc.vector.tensor_scalar(
                    out=tmp2, in0=xt[:, di:di + H, dj:dj + W],
                    scalar1=ks[:, t:t + 1], scalar2=None, op0=OP.mult)
                nc.vector.tensor_add(out=acc, in0=acc, in1=tmp2)
            for tmp in tmps:
                nc.vector.tensor_add(out=acc, in0=acc, in1=tmp)
            nc.vector.tensor_add(out=acc, in0=acc, in1=ev)
            nc.vector.tensor_add(out=acc, in0=acc, in1=accg)

            nc.sync.dma_start(out=outd[b, g * P:(g + 1) * P, :, :], in_=acc)
            # DVE observer of the store completion: collapses the final
            # drain's (and slot-recycling) waits onto the DVE semaphore
            nc.vector.memset(acc[:, 0, 0:1], 0.0)

        # ---- schedule: pool/stats of b+1 interleaved into conv of b ----
        for g in range(CG):
            pool_tile(0, g)
        stats(0)
        for b in range(BL):
            for g in range(CG):
                filler = None
                if b + 1 < BL:
                    bn, gn = b + 1, g
                    def filler(bn=bn, gn=gn):
                        pool_tile(bn, gn)
                        if gn == CG - 1:
                            stats(bn)
                conv_tile(b, g, filler)

    import sys
    sys.setrecursionlimit(100000)
    _prune_redundant_waits(nc)
    return nc


_NC_CACHE = {}


def _get_nc():
    if "nc" not in _NC_CACHE:
        _NC_CACHE["nc"] = _build_bass()
    return _NC_CACHE["nc"]


def _prep_inputs(x, w_avg, b_avg, w_max, b_max, w_mix, gamma, beta):
    import ml_dtypes
    bf16 = ml_dtypes.bfloat16

    x = np.asarray(x, dtype=np.float32)
    xpad = np.zeros((B, C, HP, WP), dtype=np.float32)
    xpad[:, :, PAD:PAD + H, PAD:PAD + W] = x
    xpad = xpad.astype(bf16)

    w_avg = np.asarray(w_avg, np.float32) / (H * W)   # fold the mean
    w_max = np.asarray(w_max, np.float32)
    # stationary layout [P, 2, CG, C]: w2t[p, s, g, :] = W_s[:, g*P+p]
    w2t = np.stack([
        w_avg.T.reshape(CG, P, C).transpose(1, 0, 2),
        w_max.T.reshape(CG, P, C).transpose(1, 0, 2),
    ], axis=1)
    bsum = (np.asarray(b_avg, np.float32) + np.asarray(b_max, np.float32))
    w_mix = np.asarray(w_mix, np.float32)
    gamma = np.asarray(gamma, np.float32).reshape(C)
    beta = np.asarray(beta, np.float32).reshape(C)
    gw = (gamma[:, None] * w_mix).reshape(CG, P, NTAPS).transpose(1, 0, 2)
    wn = np.sqrt((w_mix * w_mix).sum(axis=1)).reshape(CG, P).T
    istk = np.eye(P, dtype=np.float32).astype(bf16)

    shared = {
        "w2t": np.ascontiguousarray(w2t),
        "bsum": np.ascontiguousarray(bsum.reshape(CG, P).T),
        "gw": np.ascontiguousarray(gw),
        "wn": np.ascontiguousarray(wn),
        "bet": np.ascontiguousarray(beta.reshape(CG, P).T),
        "istk": istk,
    }
    in_maps = []
    for i in range(NCORES):
        m = dict(shared)
        m["xp"] = np.ascontiguousarray(xpad[i * BL:(i + 1) * BL])
        in_maps.append(m)
    return in_maps


def run(inputs, trace=False):
    nc = _get_nc()
    in_maps = _prep_inputs(**inputs)
    res = bass_utils.run_bass_kernel_spmd(
        nc, in_maps, core_ids=list(range(NCORES)), trace=trace)
    outs = [np.asarray(res.results[i]["out"]) for i in range(NCORES)]
    full = np.concatenate(outs, axis=0).astype(np.float32)
    return full, res


def kernel(**inputs) -> np.ndarray:
    full, _ = run(inputs, trace=False)
    return full
